# revision 22
# baseline (speedup 1.0000x reference)
"""Trainium2 Bass kernel for nn_Attention_26628797235884.

12-head attention block (qkv proj + per-head RMS norm + 2D RoPE + softmax
attention + output proj), batch 8 x seq 1024 x dim 768, data-parallel over
batch across 8 NeuronCores (batch b -> core b, weights replicated).

Per-core design (v2):
  - q,k computed feature-major ([feat, seq]) so QK^T and PV contract along
    partitions without transposes; v in natural [seq, feat] layout.
  - exp split across two engines: head-A exps on ACT (exact Exp), head-B
    exps on DVE via a Schraudolph fp16 bit-trick (x*S+B -> int16, bit-viewed
    as fp16); softmax averaging dilutes the ~1.8% rms weight error ~50x.
  - PV lhsT widened to M=128 with 64 replicated ones-columns (costs no PE
    time: matmul cost = N only) so the softmax denominator lands replicated
    on psum partitions 64:127 -> 64-lane DVE reciprocal + one mixed-base DVE
    multiply finish the epilogue (no DMA / gpsimd broadcast).
  - qkv/proj biases folded into the matmuls via K=1 ones-row passes; psum
    drains on ACT; sum-of-squares via ACT Square + indicator matmul.
  - DMA ordered so x/qkv weights land first; proj weights last.
All matmuls bf16/fp16 with fp32 accumulation.
"""

import sys

import numpy as np
import ml_dtypes

try:
    import concourse.bass as bass  # noqa: F401
except ImportError:  # pragma: no cover
    sys.path.insert(0, "/opt/trn_rl_repo")

import concourse.tile as tile
from concourse import bacc, mybir
from concourse.bass_utils import run_bass_kernel_spmd

BF16 = mybir.dt.bfloat16
F16 = mybir.dt.float16
I16 = mybir.dt.int16
F32 = mybir.dt.float32
AF = mybir.ActivationFunctionType
OP = mybir.AluOpType
NP_BF16 = ml_dtypes.bfloat16

B, S, C, H, D = 8, 1024, 768, 12, 64
KT = C // 128          # 6 contraction tiles over the model dim
ST = S // 128          # 8 seq tiles
NCORES = 8
EPS = 1e-6
EPS_V = 1e-6
PAIRSWAP32 = [i ^ 1 for i in range(32)]

# bf16 Schraudolph exp: bits16 = trunc(x*0.125*128*log2e + (127*128+0.5-C))
# (bf16 not fp16: fp16 matmuls stream at half the bf16 rate on TRN2 HW)
SC_EXP = 128.0 * 1.4426950408889634 * 0.125
BC_EXP = 127.0 * 128.0 + 0.5 - 7.40

_CACHE = {}


# --------------------------------------------------------------------------
# host-side constant prep
# --------------------------------------------------------------------------

def _rope_tables():
    ROPE_DIM, PT_SEQ, FT_SEQ, THETA = 32, 16, 32, 10000.0
    freqs = 1.0 / (THETA ** (np.arange(0, ROPE_DIM, 2, dtype=np.float32)[: ROPE_DIM // 2] / ROPE_DIM))
    t = np.arange(FT_SEQ, dtype=np.float32) / FT_SEQ * PT_SEQ
    f = np.einsum("i,j->ij", t, freqs)
    f = np.repeat(f, 2, axis=-1)
    fh = np.broadcast_to(f[:, None, :], (FT_SEQ, FT_SEQ, ROPE_DIM))
    fw = np.broadcast_to(f[None, :, :], (FT_SEQ, FT_SEQ, ROPE_DIM))
    f2 = np.concatenate([fh, fw], axis=-1).reshape(FT_SEQ * FT_SEQ, 2 * ROPE_DIM)
    return np.cos(f2).astype(np.float32), np.sin(f2).astype(np.float32)


def _prep_shared(qkv_w, qkv_b, q_norm_w, k_norm_w, proj_w, proj_b):
    f32 = np.float32
    cos, sin = _rope_tables()                 # [S, D]
    pair = np.arange(D) ^ 1
    sa = sin.copy()
    sa[:, 0::2] *= -1.0                       # sign-folded sin for rotate_half

    def mk(tab, w):                           # -> [128, S] bf16, 2 heads stacked
        t = (tab * w[None, :]).T.astype(f32)  # [64, S]
        return np.ascontiguousarray(np.vstack([t, t])).astype(NP_BF16)

    qw = np.asarray(q_norm_w, f32)
    kw = np.asarray(k_norm_w, f32)
    shared = {
        "wqkT": np.ascontiguousarray(np.asarray(qkv_w, f32)[: 2 * C].T).astype(NP_BF16),
        "wvT": np.ascontiguousarray(np.asarray(qkv_w, f32)[2 * C :].T).astype(NP_BF16),
        "pwT": np.ascontiguousarray(np.asarray(proj_w, f32).T).astype(NP_BF16),
        "cosq": mk(cos, qw),
        "sinq": mk(sa, qw[pair]),
        "cosk": mk(cos, kw),
        "sink": mk(sa, kw[pair]),
    }
    b = np.asarray(qkv_b, f32)
    shared["bqk"] = np.ascontiguousarray(b[: 2 * C].reshape(2 * KT, 128).T)  # [128, 12]
    shared["vbias"] = np.ascontiguousarray(b[2 * C :][None, :]).astype(NP_BF16)       # [1, 768]
    shared["pbias"] = np.ascontiguousarray(np.asarray(proj_b, f32)[None, :]).astype(NP_BF16)
    # E_big[:, 10 - 2i : 22 - 2i] is a [128, 12] indicator lhsT whose column
    # 2i+g selects partition half g — lets 6 m-tiles' group-sums accumulate
    # into one [12, S] PSUM tensor.
    ebig = np.zeros((128, 22), NP_BF16)
    ebig[0:64, 10] = 1
    ebig[64:128, 11] = 1
    shared["ebig"] = ebig
    # sel[:, 128i:128i+128] broadcasts rinv rows (2i, 2i+1) to the 2 head
    # halves of a [128, S] field via lhsT.T @ rinv.
    sel = np.zeros((12, 6 * 128), NP_BF16)
    for i in range(6):
        sel[2 * i, 128 * i : 128 * i + 64] = 1
        sel[2 * i + 1, 128 * i + 64 : 128 * i + 128] = 1
    shared["sel"] = sel
    shared["ident6"] = np.eye(6, dtype=np.float32)
    shared["epsc"] = np.array([[EPS_V]] * 6 + [[64.0 * EPS_V]] * 6, dtype=np.float32)
    shared["sqscale"] = np.array([[1.0 / 64.0]] * 6 + [[1.0]] * 6, dtype=np.float32)
    return shared


# --------------------------------------------------------------------------
# device graph
# --------------------------------------------------------------------------

def _graph(tc, d, out_d):
    nc = tc.nc
    from contextlib import ExitStack

    with ExitStack() as big:
        main = big.enter_context(tc.tile_pool(name="main", bufs=1))

        qk_sb = [main.tile([128, S], BF16, tag=f"qk{m}", name=f"qk{m}") for m in range(2 * KT)]
        v_sb = [main.tile([128, H * 128], BF16, tag=f"v{j}", name=f"v{j}") for j in range(ST)]
        outT = [main.tile([128, S], BF16, tag=f"ot{p}", name=f"ot{p}") for p in range(KT)]
        ones_r = main.tile([1, 128], BF16, tag="ones_r")
        rk_act = main.tile([128, 96], F32, tag="rk_act")
        rk_dve = main.tile([128, 96], F32, tag="rk_dve")
        pwT = main.tile([128, KT, C], BF16, tag="pwT")
        pbias = main.tile([1, C], BF16, tag="pbias")

        # ---------------- stage 1+2: projections, norm, rope ----------------
        with ExitStack() as early:
            ep = early.enter_context(tc.tile_pool(name="early", bufs=1))
            w1 = early.enter_context(tc.tile_pool(name="w1", bufs=2))
            w1b = early.enter_context(tc.tile_pool(name="w1b", bufs=6))
            t1p = early.enter_context(tc.tile_pool(name="t1p", bufs=2))
            qkv_ps = early.enter_context(ExitStack())
            ps_mm = qkv_ps.enter_context(tc.tile_pool(name="psmm", bufs=3, space="PSUM"))
            ps_sq = qkv_ps.enter_context(tc.tile_pool(name="pssq", bufs=1, space="PSUM"))
            ps_fld = qkv_ps.enter_context(tc.tile_pool(name="psfld", bufs=2, space="PSUM"))

            # DMA order = need order: x + qkv weights, tables, v weights,
            # proj weights last. Coarse chunks: descriptor generation costs
            # ~0.6us of queue time per dma_start, so fewer is faster.
            xT = ep.tile([128, KT, S], BF16, tag="xT")
            xT_r = d["xT"].rearrange("(k p) s -> p k s", p=128)
            wqkT = ep.tile([128, KT, 2 * C], BF16, tag="wqkT")
            wqkT_r = d["wqkT"].rearrange("(k p) o -> p k o", p=128)
            wvT = ep.tile([128, KT, C], BF16, tag="wvT")
            wvT_r = d["wvT"].rearrange("(k p) o -> p k o", p=128)
            tabs = {}
            for nm in ("cosq", "sinq", "cosk", "sink"):
                tabs[nm] = ep.tile([128, S], BF16, tag=nm, name=nm)
            bqk = ep.tile([128, 2 * KT], F32, tag="bqk")
            for k in range(KT):
                nc.sync.dma_start(xT[:, k], xT_r[:, k])
                nc.scalar.dma_start(wqkT[:, k, 0:1024], wqkT_r[:, k, 0:1024])
                nc.gpsimd.dma_start(wqkT[:, k, 1024:1536], wqkT_r[:, k, 1024:1536])
            nc.sync.dma_start(bqk[:], d["bqk"][:])
            for nm in ("cosq", "sinq", "cosk", "sink"):
                nc.sync.dma_start(tabs[nm][:], d[nm][:])
            ebig = ep.tile([128, 22], BF16, tag="ebig")
            nc.sync.dma_start(ebig[:], d["ebig"][:])
            sel = ep.tile([12, 6 * 128], BF16, tag="sel")
            nc.sync.dma_start(sel[:], d["sel"][:])
            ident6 = ep.tile([6, 6], F32, tag="ident6")
            nc.sync.dma_start(ident6[:], d["ident6"][:])
            epsc = ep.tile([12, 1], F32, tag="epsc")
            nc.sync.dma_start(epsc[:], d["epsc"][:])
            sqscale = ep.tile([12, 1], F32, tag="sqscale")
            nc.sync.dma_start(sqscale[:], d["sqscale"][:])
            for k in range(KT):
                nc.sync.dma_start(wvT[:, k], wvT_r[:, k])
            vbias = ep.tile([1, C], BF16, tag="vbias")
            nc.sync.dma_start(vbias[:], d["vbias"][:])
            pwT_r = d["pwT"].rearrange("(k p) o -> p k o", p=128)
            for k in range(KT):
                nc.sync.dma_start(pwT[:, k], pwT_r[:, k])
            nc.sync.dma_start(pbias[:], d["pbias"][:])

            nc.gpsimd.memset(ones_r[:], 1.0)
            for j in range(ST):
                # ones-columns 0:64 of every head: denominator replication.
                # Ones FIRST so Z lands on psum partitions 0:63 —
                # reciprocal_approx_fast at partition base 64 no-ops on HW.
                nc.gpsimd.memset(
                    v_sb[j][:].rearrange("p (h e) -> p h e", e=128)[:, :, 0:64], 1.0
                )

            # q-side rinv is applied to qk_sb via sel-matmul broadcast; the
            # k-side rinv (x0.125 softmax scale) is instead folded into the
            # exp scale per sk-partition in stage 3, so k-side rope output
            # goes straight into qk_sb and needs no field multiply.
            batches = [[0, 1, 2, 6, 7, 8], [3, 4, 5, 9, 10, 11]]
            pending_tails = []
            for batch, ms in enumerate(batches):
                nb = len(ms)
                sqb = ps_sq.tile([2 * nb, S], F32, tag="sq", name=f"sqb{batch}")
                t1s = []
                for i, m in enumerate(ms):
                    if batch >= 1 and i == 2 and pending_tails:
                        pending_tails.pop(0)()  # previous batch's fields
                    ctab = tabs["cosq"] if m < KT else tabs["cosk"]
                    stab = tabs["sinq"] if m < KT else tabs["sink"]
                    if m < KT:
                        t1 = t1p.tile([128, S], BF16, tag=f"t1_{i}", name=f"t1_{batch}_{i}")
                        t1s.append(t1)
                    else:
                        t1 = qk_sb[m]
                    for h2 in range(2):
                        cs = slice(512 * h2, 512 * h2 + 512)
                        ps = ps_mm.tile([128, 512], F32, tag="mm", name=f"mm{batch}_{i}_{h2}")
                        for k in range(KT):
                            nc.tensor.matmul(
                                ps[:],
                                wqkT[:, k, 128 * m : 128 * m + 128],
                                xT[:, k, cs],
                                start=(k == 0),
                                stop=(k == KT - 1),
                            )
                        # two ACT readers of the psum: t = ps + b, t2 = (ps + b)^2
                        t = w1b.tile([128, 512], BF16, tag="t")
                        nc.scalar.activation(t[:], ps[:], AF.Identity, bias=bqk[:, m : m + 1], scale=1.0)
                        t2 = w1b.tile([128, 512], BF16, tag="t2")
                        nc.scalar.activation(t2[:], ps[:], AF.Square, bias=bqk[:, m : m + 1], scale=1.0)
                        nc.tensor.matmul(
                            sqb[:, cs],
                            ebig[:, 10 - 2 * i : 10 - 2 * i + 2 * nb],
                            t2[:],
                            start=(i == 0), stop=(i == nb - 1),
                        )
                        # rope: u = t*cos ; vv = shuffle(t)*sinA
                        u = w1b.tile([128, 512], BF16, tag="u")
                        nc.vector.tensor_mul(u[:], t[:], ctab[:, cs])
                        tsh = w1b.tile([128, 512], BF16, tag="tsh")
                        nc.vector.stream_shuffle(tsh[:], t[:], PAIRSWAP32)
                        vv = w1b.tile([128, 512], BF16, tag="vv")
                        nc.vector.tensor_mul(vv[:], tsh[:], stab[:, cs])
                        nc.gpsimd.tensor_add(t1[:, cs], u[:], vv[:])
                # rows 0:6 = q-side 1/sqrt(ssq/64+eps); rows 6:12 = k-side
                # with the 0.125 softmax scale folded in for free via
                # 1/sqrt(ssq + 64*eps) = 0.125 / sqrt(ssq/64 + eps).
                # (epsc/sqscale come from DRAM: only their rows differ.)
                rms = w1.tile([2 * nb, S], F32, tag="rms", name=f"rms{batch}")
                nc.scalar.activation(rms[:], sqb[:], AF.Sqrt, bias=epsc[:], scale=sqscale[:])
                rinv = w1.tile([2 * nb, S], F32, tag="rinv", name=f"rinv{batch}")
                nc.vector.reciprocal_approx_fast(rinv[:], rms[:])
                rinv_bf = w1.tile([6, S], BF16, tag="rinv_bf", name=f"rinvbf{batch}")
                nc.vector.tensor_copy(rinv_bf[:], rinv[0:6, :])
                # k rows down to partition base 0 (DMA is the reliable way to
                # move across partitions), then PE-transpose to [sk, head*j]
                rkt0 = w1.tile([6, S], F32, tag="rkt0", name=f"rkt0{batch}")
                nc.sync.dma_start(rkt0[:], rinv[6:12, :])
                tp = ps_sq.tile([128, 48], F32, tag="tp", name=f"tp{batch}")
                for j in range(ST):
                    nc.tensor.transpose(tp[:, 6 * j : 6 * j + 6],
                                        rkt0[:, 128 * j : 128 * j + 128], ident6[:])
                nc.scalar.activation(rk_act[:, 48 * batch : 48 * batch + 48], tp[:], AF.Identity, scale=1.0)
                nc.scalar.activation(rk_dve[:, 48 * batch : 48 * batch + 48], tp[:], AF.Identity,
                                     scale=128.0 * 1.4426950408889634)

                def _mk_tail(ms=ms, t1s=t1s, rinv_bf=rinv_bf):
                    def _tail():
                        for i, m in enumerate(ms[0:3]):
                            for h2 in range(2):
                                cs = slice(512 * h2, 512 * h2 + 512)
                                fldp = ps_fld.tile([128, 512], F32, tag="fld", name=f"fld{m}_{h2}")
                                nc.tensor.matmul(
                                    fldp[:],
                                    sel[0:6, 128 * i : 128 * i + 128],
                                    rinv_bf[:, cs],
                                    start=True, stop=True,
                                )
                                nc.vector.tensor_mul(qk_sb[m][:, cs], t1s[i][:, cs], fldp[:])
                    return _tail
                pending_tails.append(_mk_tail())

            for t in pending_tails:
                t()
            pending_tails = []

            # ------------- stage 2: V projection (own psum scope) -----------
            # Dense PE work here overlaps the last batch's DVE/Pool tails.
            qkv_ps.close()
            ps_v = early.enter_context(tc.tile_pool(name="psv", bufs=2, space="PSUM"))
            for j in range(ST):
                vview = v_sb[j][:].rearrange("p (h e) -> p h e", e=128)
                for lo, cw, nh in ((0, 512, 8), (512, 256, 4)):
                    psv = ps_v.tile([128, cw], F32, tag=f"vmm{cw}", name=f"vmm{j}_{lo}")
                    for k in range(KT):
                        nc.tensor.matmul(
                            psv[:], xT[:, k, 128 * j : 128 * j + 128],
                            wvT[:, k, lo : lo + cw],
                            start=(k == 0), stop=False,
                        )
                    nc.tensor.matmul(
                        psv[:], ones_r[:], vbias[:, lo : lo + cw],
                        start=False, stop=True,
                    )
                    nc.scalar.activation(
                        vview[:, lo // 64 : lo // 64 + nh, 64:128], psv[:], AF.Identity, scale=1.0
                    )

        # ---------------- stage 3: attention, software-pipelined ------------
        # PV is organized as per-(head, q-half) accumulation groups over all
        # j (no exp dependencies once the pair's exps exist) and interleaved
        # between the NEXT pair's score matmuls, so the PE has dense filler
        # work instead of chain-waiting on exp slot recycling.
        with ExitStack() as att:
            xpa = att.enter_context(tc.tile_pool(name="attxa", bufs=12))
            xpb = att.enter_context(tc.tile_pool(name="attxb", bufs=12))
            rip = att.enter_context(tc.tile_pool(name="attri", bufs=3))
            ps_sc = att.enter_context(tc.tile_pool(name="pssc", bufs=3, space="PSUM"))
            ps_pv = att.enter_context(tc.tile_pool(name="pspv", bufs=2, space="PSUM"))

            def mk_pv_chunks(p, eAs, eBs):
                """8 closures: two per (head, q-half) group; the second also
                emits the group's recip+normalize epilogue."""
                chunks = []
                for g in range(4):
                    hh, qh = g // 2, g % 2
                    cs = slice(512 * qh, 512 * qh + 512)
                    es = eAs if hh == 0 else eBs
                    h = 2 * p + hh
                    box = {}

                    def first(box=box, h=h, cs=cs, es=es, p=p, g=g):
                        pv = ps_pv.tile([128, 512], F32, tag="pv", name=f"pv{p}_{g}")
                        box["pv"] = pv
                        for j in range(4):
                            vva = v_sb[j][:].rearrange("p (h e) -> p h e", e=128)
                            nc.tensor.matmul(pv[:], vva[:, h, :], es[j][:, cs],
                                             start=(j == 0), stop=False)

                    def second(box=box, h=h, cs=cs, es=es, hh=hh, p=p, g=g):
                        pv = box["pv"]
                        for j in range(4, ST):
                            vva = v_sb[j][:].rearrange("p (h e) -> p h e", e=128)
                            nc.tensor.matmul(pv[:], vva[:, h, :], es[j][:, cs],
                                             start=False, stop=(j == ST - 1))
                        # denominators replicated on psum rows 0:63 via the
                        # leading ones-columns of v; dims on rows 64:127.
                        ri = rip.tile([64, 512], F32, tag="ri", name=f"ri{p}_{g}")
                        nc.vector.reciprocal_approx_fast(ri[:], pv[0:64, :])
                        rows = slice(0, 64) if hh == 0 else slice(64, 128)
                        nc.vector.tensor_mul(outT[p][rows, cs], pv[64:128, :], ri[:])

                    chunks.append(first)
                    chunks.append(second)
                return chunks

            pending_pv = []
            for p in range(KT):
                qt, kt = qk_sb[p], qk_sb[KT + p]
                eAs, eBs = [], []
                for j in range(ST):
                    scA = ps_sc.tile([128, S], F32, tag="sc", name=f"scA{p}_{j}")
                    scB = ps_sc.tile([128, S], F32, tag="sc", name=f"scB{p}_{j}")
                    for h2 in range(2):
                        cs = slice(512 * h2, 512 * h2 + 512)
                        nc.tensor.matmul(
                            scA[:, cs],
                            kt[0:64, 128 * j : 128 * j + 128], qt[0:64, cs],
                            start=True, stop=True,
                        )
                        nc.tensor.matmul(
                            scB[:, cs],
                            kt[64:128, 128 * j : 128 * j + 128], qt[64:128, cs],
                            start=True, stop=True,
                        )
                    # exp split: head A exact on ACT; head B Schraudolph on
                    # DVE (bf16 bit-trick), except 2 of 8 j on ACT to balance.
                    # The per-sk-partition scale = 0.125 * rinv_k (rms norm of
                    # k folded into the softmax scale).
                    cA = 48 * (p // 3) + 6 * j + (2 * p) % 6
                    eA = xpa.tile([128, S], BF16, tag="expA", name=f"eA{p}_{j}")
                    nc.scalar.activation(eA[:], scA[:], AF.Exp, scale=rk_act[:, cA : cA + 1])
                    eB = xpb.tile([128, S], BF16, tag="expB", name=f"eB{p}_{j}")
                    if j in (3, 7):
                        nc.scalar.activation(eB[:], scB[:], AF.Exp, scale=rk_act[:, cA + 1 : cA + 2])
                    else:
                        nc.vector.tensor_scalar(eB[:].bitcast(I16), scB[:],
                                                rk_dve[:, cA + 1 : cA + 2], BC_EXP, OP.mult, OP.add)
                    eAs.append(eA)
                    eBs.append(eB)
                    if pending_pv:
                        pending_pv.pop(0)()
                pending_pv.extend(mk_pv_chunks(p, eAs, eBs))
            for c in pending_pv:
                c()

        # ---------------- stage 4: output projection ------------------------
        with ExitStack() as late:
            yp = late.enter_context(tc.tile_pool(name="yp", bufs=2))
            ps_y = late.enter_context(tc.tile_pool(name="psy", bufs=2, space="PSUM"))
            for mt in range(ST):
                ps = ps_y.tile([128, C], F32, tag="y")
                for cl, cw in ((0, 512), (512, 256)):
                    for k6 in range(KT):
                        nc.tensor.matmul(
                            ps[:, cl : cl + cw],
                            outT[k6][:, 128 * mt : 128 * mt + 128],
                            pwT[:, k6, cl : cl + cw],
                            start=(k6 == 0), stop=False,
                        )
                    nc.tensor.matmul(
                        ps[:, cl : cl + cw], ones_r[:], pbias[:, cl : cl + cw],
                        start=False, stop=True,
                    )
                y = yp.tile([128, C], F32, tag="y_sb")
                nc.scalar.activation(y[:], ps[:], AF.Identity, scale=1.0)
                nc.sync.dma_start(out_d[128 * mt : 128 * mt + 128, :], y[:])


LDW_OPT = False  # walrus LDW-opt rejects bass InstLdweights


def _patch_walrus():
    import concourse.bass_utils as _bu
    if getattr(_bu, "_ldwopt_patched", False):
        return
    _orig = _bu.run_command

    def _patched(cmd, **kw):
        if LDW_OPT and isinstance(cmd, list):
            cmd = ["--enable-ldw-opt=true" if c == "--enable-ldw-opt=false" else c for c in cmd]
        return _orig(cmd, **kw)

    _bu.run_command = _patched
    _bu._ldwopt_patched = True


def build():
    if "nc" in _CACHE:
        return _CACHE["nc"]
    _patch_walrus()
    nc = bacc.Bacc("TRN2", target_bir_lowering=False, debug=False)
    d = {}

    def din(name, shape, dt):
        d[name] = nc.dram_tensor(name, shape, dt, kind="ExternalInput").ap()

    din("xT", [C, S], BF16)
    din("wqkT", [C, 2 * C], BF16)
    din("wvT", [C, C], BF16)
    din("pwT", [C, C], BF16)
    din("bqk", [128, 2 * KT], F32)
    din("vbias", [1, C], BF16)
    din("pbias", [1, C], BF16)
    din("cosq", [128, S], BF16)
    din("sinq", [128, S], BF16)
    din("cosk", [128, S], BF16)
    din("sink", [128, S], BF16)
    din("ebig", [128, 22], BF16)
    din("ident6", [6, 6], F32)
    din("epsc", [12, 1], F32)
    din("sqscale", [12, 1], F32)
    din("sel", [12, 6 * 128], BF16)
    out_d = nc.dram_tensor("out", [S, C], F32, kind="ExternalOutput").ap()

    with tile.TileContext(nc) as tc:
        _graph(tc, d, out_d)
    nc.compile()
    _CACHE["nc"] = nc
    return nc


def make_in_maps(x, qkv_w, qkv_b, q_norm_w, k_norm_w, proj_w, proj_b):
    shared = _prep_shared(qkv_w, qkv_b, q_norm_w, k_norm_w, proj_w, proj_b)
    x = np.asarray(x, np.float32)
    in_maps = []
    for b in range(NCORES):
        m = dict(shared)
        m["xT"] = np.ascontiguousarray(x[b].T).astype(NP_BF16)
        in_maps.append(m)
    return in_maps


def run(in_maps, trace=False, **kw):
    nc = build()
    return run_bass_kernel_spmd(nc, in_maps, core_ids=list(range(NCORES)), trace=trace, **kw)


def kernel(x, qkv_w, qkv_b, q_norm_w, k_norm_w, proj_w, proj_b):
    in_maps = make_in_maps(x, qkv_w, qkv_b, q_norm_w, k_norm_w, proj_w, proj_b)
    res = run(in_maps)
    return np.stack([np.asarray(res.results[i]["out"]) for i in range(NCORES)]).astype(np.float32)


if __name__ == "__main__":
    rng = np.random.default_rng(0)
    ins = {
        "x": rng.standard_normal((B, S, C)).astype(np.float32),
        "qkv_w": (rng.standard_normal((3 * C, C)) * C**-0.5).astype(np.float32),
        "qkv_b": (rng.standard_normal(3 * C) * 0.02).astype(np.float32),
        "q_norm_w": np.ones(D, np.float32),
        "k_norm_w": np.ones(D, np.float32),
        "proj_w": (rng.standard_normal((C, C)) * C**-0.5).astype(np.float32),
        "proj_b": (rng.standard_normal(C) * 0.02).astype(np.float32),
    }
    y = kernel(**ins)
    print("out", y.shape, y.dtype)


# revision 23
# speedup vs baseline: 1.1725x; 1.1725x over previous
"""Trainium2 Bass kernel for nn_Attention_26628797235884.

12-head attention block (qkv proj + per-head RMS norm + 2D RoPE + softmax
attention + output proj), batch 8 x seq 1024 x dim 768, data-parallel over
batch across 8 NeuronCores (batch b -> core b, weights replicated).

Per-core design (v2):
  - q,k computed feature-major ([feat, seq]) so QK^T and PV contract along
    partitions without transposes; v in natural [seq, feat] layout.
  - exp split across two engines: head-A exps on ACT (exact Exp), head-B
    exps on DVE via a Schraudolph fp16 bit-trick (x*S+B -> int16, bit-viewed
    as fp16); softmax averaging dilutes the ~1.8% rms weight error ~50x.
  - PV lhsT widened to M=128 with 64 replicated ones-columns (costs no PE
    time: matmul cost = N only) so the softmax denominator lands replicated
    on psum partitions 64:127 -> 64-lane DVE reciprocal + one mixed-base DVE
    multiply finish the epilogue (no DMA / gpsimd broadcast).
  - qkv/proj biases folded into the matmuls via K=1 ones-row passes; psum
    drains on ACT; sum-of-squares via ACT Square + indicator matmul.
  - DMA ordered so x/qkv weights land first; proj weights last.
All matmuls bf16/fp16 with fp32 accumulation.
"""

import sys

import numpy as np
import ml_dtypes

try:
    import concourse.bass as bass  # noqa: F401
except ImportError:  # pragma: no cover
    sys.path.insert(0, "/opt/trn_rl_repo")

import concourse.tile as tile
from concourse import bacc, mybir
from concourse.bass_utils import run_bass_kernel_spmd

BF16 = mybir.dt.bfloat16
F16 = mybir.dt.float16
I16 = mybir.dt.int16
F32 = mybir.dt.float32
AF = mybir.ActivationFunctionType
OP = mybir.AluOpType
NP_BF16 = ml_dtypes.bfloat16

B, S, C, H, D = 8, 1024, 768, 12, 64
KT = C // 128          # 6 contraction tiles over the model dim
ST = S // 128          # 8 seq tiles
NCORES = 8
EPS = 1e-6
EPS_V = 1e-6
PAIRSWAP32 = [i ^ 1 for i in range(32)]

# bf16 Schraudolph exp: bits16 = trunc(x*0.125*128*log2e + (127*128+0.5-C))
# (bf16 not fp16: fp16 matmuls stream at half the bf16 rate on TRN2 HW)
SC_EXP = 128.0 * 1.4426950408889634 * 0.125
BC_EXP = 127.0 * 128.0 + 0.5 - 7.40

_CACHE = {}


# --------------------------------------------------------------------------
# host-side constant prep
# --------------------------------------------------------------------------

def _rope_tables():
    ROPE_DIM, PT_SEQ, FT_SEQ, THETA = 32, 16, 32, 10000.0
    freqs = 1.0 / (THETA ** (np.arange(0, ROPE_DIM, 2, dtype=np.float32)[: ROPE_DIM // 2] / ROPE_DIM))
    t = np.arange(FT_SEQ, dtype=np.float32) / FT_SEQ * PT_SEQ
    f = np.einsum("i,j->ij", t, freqs)
    f = np.repeat(f, 2, axis=-1)
    fh = np.broadcast_to(f[:, None, :], (FT_SEQ, FT_SEQ, ROPE_DIM))
    fw = np.broadcast_to(f[None, :, :], (FT_SEQ, FT_SEQ, ROPE_DIM))
    f2 = np.concatenate([fh, fw], axis=-1).reshape(FT_SEQ * FT_SEQ, 2 * ROPE_DIM)
    return np.cos(f2).astype(np.float32), np.sin(f2).astype(np.float32)


def _prep_shared(qkv_w, qkv_b, q_norm_w, k_norm_w, proj_w, proj_b):
    f32 = np.float32
    cos, sin = _rope_tables()                 # [S, D]
    pair = np.arange(D) ^ 1
    sa = sin.copy()
    sa[:, 0::2] *= -1.0                       # sign-folded sin for rotate_half

    def mk(tab, w):                           # -> [128, S] bf16, 2 heads stacked
        t = (tab * w[None, :]).T.astype(f32)  # [64, S]
        return np.ascontiguousarray(np.vstack([t, t])).astype(NP_BF16)

    qw = np.asarray(q_norm_w, f32)
    kw = np.asarray(k_norm_w, f32)
    shared = {
        "wqkT": np.ascontiguousarray(np.asarray(qkv_w, f32)[: 2 * C].T).astype(NP_BF16),
        "wvT": np.ascontiguousarray(np.asarray(qkv_w, f32)[2 * C :].T).astype(NP_BF16),
        "pwT": np.ascontiguousarray(np.asarray(proj_w, f32).T).astype(NP_BF16),
        "cosq": mk(cos, qw),
        "sinq": mk(sa, qw[pair]),
        "cosk": mk(cos, kw),
        "sink": mk(sa, kw[pair]),
    }
    b = np.asarray(qkv_b, f32)
    shared["bqk"] = np.ascontiguousarray(b[: 2 * C].reshape(2 * KT, 128).T)  # [128, 12]
    shared["vbias"] = np.ascontiguousarray(b[2 * C :][None, :]).astype(NP_BF16)       # [1, 768]
    shared["pbias"] = np.ascontiguousarray(np.asarray(proj_b, f32)[None, :]).astype(NP_BF16)
    # E_big[:, 10 - 2i : 22 - 2i] is a [128, 12] indicator lhsT whose column
    # 2i+g selects partition half g — lets 6 m-tiles' group-sums accumulate
    # into one [12, S] PSUM tensor.
    ebig = np.zeros((128, 22), NP_BF16)
    ebig[0:64, 10] = 1
    ebig[64:128, 11] = 1
    shared["ebig"] = ebig
    # sel[:, 128i:128i+128] broadcasts rinv rows (2i, 2i+1) to the 2 head
    # halves of a [128, S] field via lhsT.T @ rinv.
    sel = np.zeros((12, 6 * 128), NP_BF16)
    for i in range(6):
        sel[2 * i, 128 * i : 128 * i + 64] = 1
        sel[2 * i + 1, 128 * i + 64 : 128 * i + 128] = 1
    shared["sel"] = sel
    shared["epsc"] = np.full((12, 1), EPS_V, dtype=np.float32)
    return shared


# --------------------------------------------------------------------------
# device graph
# --------------------------------------------------------------------------

def _graph(tc, d, out_d):
    nc = tc.nc
    from contextlib import ExitStack

    with ExitStack() as big:
        main = big.enter_context(tc.tile_pool(name="main", bufs=1))

        qk_sb = [main.tile([128, S], BF16, tag=f"qk{m}", name=f"qk{m}") for m in range(2 * KT)]
        v_sb = [main.tile([128, H * 128], BF16, tag=f"v{j}", name=f"v{j}") for j in range(ST)]
        outT = [main.tile([128, S], BF16, tag=f"ot{p}", name=f"ot{p}") for p in range(KT)]
        ones_r = main.tile([1, 128], BF16, tag="ones_r")
        pwT = main.tile([128, KT, C], BF16, tag="pwT")
        pbias = main.tile([1, C], BF16, tag="pbias")

        # ---------------- stage 1+2: projections, norm, rope ----------------
        with ExitStack() as early:
            ep = early.enter_context(tc.tile_pool(name="early", bufs=1))
            w1 = early.enter_context(tc.tile_pool(name="w1", bufs=2))
            w1b = early.enter_context(tc.tile_pool(name="w1b", bufs=6))
            t1p = early.enter_context(tc.tile_pool(name="t1p", bufs=2))
            qkv_ps = early.enter_context(ExitStack())
            ps_mm = qkv_ps.enter_context(tc.tile_pool(name="psmm", bufs=3, space="PSUM"))
            ps_sq = qkv_ps.enter_context(tc.tile_pool(name="pssq", bufs=1, space="PSUM"))
            ps_fld = qkv_ps.enter_context(tc.tile_pool(name="psfld", bufs=2, space="PSUM"))

            # DMA order = need order: x + qkv weights, tables, v weights,
            # proj weights last. Coarse chunks: descriptor generation costs
            # ~0.6us of queue time per dma_start, so fewer is faster.
            xT = ep.tile([128, KT, S], BF16, tag="xT")
            xT_r = d["xT"].rearrange("(k p) s -> p k s", p=128)
            wqkT = ep.tile([128, KT, 2 * C], BF16, tag="wqkT")
            wqkT_r = d["wqkT"].rearrange("(k p) o -> p k o", p=128)
            wvT = ep.tile([128, KT, C], BF16, tag="wvT")
            wvT_r = d["wvT"].rearrange("(k p) o -> p k o", p=128)
            tabs = {}
            for nm in ("cosq", "sinq", "cosk", "sink"):
                tabs[nm] = ep.tile([128, S], BF16, tag=nm, name=nm)
            bqk = ep.tile([128, 2 * KT], F32, tag="bqk")
            for k in range(KT):
                nc.sync.dma_start(xT[:, k], xT_r[:, k])
                nc.scalar.dma_start(wqkT[:, k, 0:1024], wqkT_r[:, k, 0:1024])
                nc.gpsimd.dma_start(wqkT[:, k, 1024:1536], wqkT_r[:, k, 1024:1536])
            nc.sync.dma_start(bqk[:], d["bqk"][:])
            for nm in ("cosq", "sinq", "cosk", "sink"):
                nc.sync.dma_start(tabs[nm][:], d[nm][:])
            ebig = ep.tile([128, 22], BF16, tag="ebig")
            nc.sync.dma_start(ebig[:], d["ebig"][:])
            sel = ep.tile([12, 6 * 128], BF16, tag="sel")
            nc.sync.dma_start(sel[:], d["sel"][:])
            epsc = ep.tile([12, 1], F32, tag="epsc")
            nc.sync.dma_start(epsc[:], d["epsc"][:])
            for k in range(KT):
                nc.sync.dma_start(wvT[:, k], wvT_r[:, k])
            vbias = ep.tile([1, C], BF16, tag="vbias")
            nc.sync.dma_start(vbias[:], d["vbias"][:])
            pwT_r = d["pwT"].rearrange("(k p) o -> p k o", p=128)
            for k in range(KT):
                nc.sync.dma_start(pwT[:, k], pwT_r[:, k])
            nc.sync.dma_start(pbias[:], d["pbias"][:])

            nc.gpsimd.memset(ones_r[:], 1.0)
            for j in range(ST):
                # ones-columns 0:64 of every head: denominator replication.
                # Ones FIRST so Z lands on psum partitions 0:63 —
                # reciprocal_approx_fast at partition base 64 no-ops on HW.
                nc.gpsimd.memset(
                    v_sb[j][:].rearrange("p (h e) -> p h e", e=128)[:, :, 0:64], 1.0
                )

            batches = [[0, 1, 2, 6, 7, 8], [3, 4, 5, 9, 10, 11]]
            pending_tails = []
            for batch, ms in enumerate(batches):
                nb = len(ms)
                sqb = ps_sq.tile([2 * nb, S], F32, tag="sq", name=f"sqb{batch}")
                t1s = []
                for i, m in enumerate(ms):
                    if batch >= 1 and i == 2 and pending_tails:
                        pending_tails.pop(0)()  # previous batch's fields
                    ctab = tabs["cosq"] if m < KT else tabs["cosk"]
                    stab = tabs["sinq"] if m < KT else tabs["sink"]
                    t1 = t1p.tile([128, S], BF16, tag=f"t1_{i}", name=f"t1_{batch}_{i}")
                    t1s.append(t1)
                    for h2 in range(2):
                        cs = slice(512 * h2, 512 * h2 + 512)
                        ps = ps_mm.tile([128, 512], F32, tag="mm", name=f"mm{batch}_{i}_{h2}")
                        for k in range(KT):
                            nc.tensor.matmul(
                                ps[:],
                                wqkT[:, k, 128 * m : 128 * m + 128],
                                xT[:, k, cs],
                                start=(k == 0),
                                stop=(k == KT - 1),
                            )
                        # two ACT readers of the psum: t = ps + b, t2 = (ps + b)^2
                        t = w1b.tile([128, 512], BF16, tag="t")
                        nc.scalar.activation(t[:], ps[:], AF.Identity, bias=bqk[:, m : m + 1], scale=1.0)
                        t2 = w1b.tile([128, 512], BF16, tag="t2")
                        nc.scalar.activation(t2[:], ps[:], AF.Square, bias=bqk[:, m : m + 1], scale=1.0)
                        nc.tensor.matmul(
                            sqb[:, cs],
                            ebig[:, 10 - 2 * i : 10 - 2 * i + 2 * nb],
                            t2[:],
                            start=(i == 0), stop=(i == nb - 1),
                        )
                        # rope: u = t*cos ; vv = shuffle(t)*sinA
                        u = w1b.tile([128, 512], BF16, tag="u")
                        nc.vector.tensor_mul(u[:], t[:], ctab[:, cs])
                        tsh = w1b.tile([128, 512], BF16, tag="tsh")
                        nc.vector.stream_shuffle(tsh[:], t[:], PAIRSWAP32)
                        vv = w1b.tile([128, 512], BF16, tag="vv")
                        nc.vector.tensor_mul(vv[:], tsh[:], stab[:, cs])
                        nc.gpsimd.tensor_add(t1[:, cs], u[:], vv[:])
                rms = w1.tile([2 * nb, S], F32, tag="rms", name=f"rms{batch}")
                nc.scalar.activation(rms[:], sqb[:], AF.Sqrt, bias=epsc[:], scale=1.0 / D)
                rinv = w1.tile([2 * nb, S], F32, tag="rinv", name=f"rinv{batch}")
                nc.vector.reciprocal_approx_fast(rinv[:], rms[:])
                rinv_bf = w1.tile([2 * nb, S], BF16, tag="rinv_bf", name=f"rinvbf{batch}")
                nc.vector.tensor_copy(rinv_bf[:], rinv[:])

                def _mk_tail(ms=ms, t1s=t1s, rinv_bf=rinv_bf, nb=nb):
                    def _tail():
                        for i, m in enumerate(ms):
                            for h2 in range(2):
                                cs = slice(512 * h2, 512 * h2 + 512)
                                fldp = ps_fld.tile([128, 512], F32, tag="fld", name=f"fld{m}_{h2}")
                                nc.tensor.matmul(
                                    fldp[:],
                                    sel[0 : 2 * nb, 128 * i : 128 * i + 128],
                                    rinv_bf[:, cs],
                                    start=True, stop=True,
                                )
                                nc.vector.tensor_mul(qk_sb[m][:, cs], t1s[i][:, cs], fldp[:])
                    return _tail
                pending_tails.append(_mk_tail())

            for t in pending_tails:
                t()
            pending_tails = []

            # ------------- stage 2: V projection (own psum scope) -----------
            # Dense PE work here overlaps the last batch's DVE/Pool tails.
            qkv_ps.close()
            ps_v = early.enter_context(tc.tile_pool(name="psv", bufs=2, space="PSUM"))
            for j in range(ST):
                vview = v_sb[j][:].rearrange("p (h e) -> p h e", e=128)
                for lo, cw, nh in ((0, 512, 8), (512, 256, 4)):
                    psv = ps_v.tile([128, cw], F32, tag=f"vmm{cw}", name=f"vmm{j}_{lo}")
                    for k in range(KT):
                        nc.tensor.matmul(
                            psv[:], xT[:, k, 128 * j : 128 * j + 128],
                            wvT[:, k, lo : lo + cw],
                            start=(k == 0), stop=False,
                        )
                    nc.tensor.matmul(
                        psv[:], ones_r[:], vbias[:, lo : lo + cw],
                        start=False, stop=True,
                    )
                    nc.scalar.activation(
                        vview[:, lo // 64 : lo // 64 + nh, 64:128], psv[:], AF.Identity, scale=1.0
                    )

        # ---------------- stage 3: attention, software-pipelined ------------
        # PV is organized as per-(head, q-half) accumulation groups over all
        # j (no exp dependencies once the pair's exps exist) and interleaved
        # between the NEXT pair's score matmuls, so the PE has dense filler
        # work instead of chain-waiting on exp slot recycling.
        with ExitStack() as att:
            xpa = att.enter_context(tc.tile_pool(name="attxa", bufs=12))
            xpb = att.enter_context(tc.tile_pool(name="attxb", bufs=12))
            rip = att.enter_context(tc.tile_pool(name="attri", bufs=3))
            ps_sc = att.enter_context(tc.tile_pool(name="pssc", bufs=3, space="PSUM"))
            ps_pv = att.enter_context(tc.tile_pool(name="pspv", bufs=2, space="PSUM"))

            def mk_pv_chunks(p, eAs, eBs):
                """8 closures: two per (head, q-half) group; the second also
                emits the group's recip+normalize epilogue."""
                chunks = []
                for g in range(4):
                    hh, qh = g // 2, g % 2
                    cs = slice(512 * qh, 512 * qh + 512)
                    es = eAs if hh == 0 else eBs
                    h = 2 * p + hh
                    box = {}

                    def first(box=box, h=h, cs=cs, es=es, p=p, g=g):
                        pv = ps_pv.tile([128, 512], F32, tag="pv", name=f"pv{p}_{g}")
                        box["pv"] = pv
                        for j in range(4):
                            vva = v_sb[j][:].rearrange("p (h e) -> p h e", e=128)
                            nc.tensor.matmul(pv[:], vva[:, h, :], es[j][:, cs],
                                             start=(j == 0), stop=False)

                    def second(box=box, h=h, cs=cs, es=es, hh=hh, p=p, g=g):
                        pv = box["pv"]
                        for j in range(4, ST):
                            vva = v_sb[j][:].rearrange("p (h e) -> p h e", e=128)
                            nc.tensor.matmul(pv[:], vva[:, h, :], es[j][:, cs],
                                             start=False, stop=(j == ST - 1))
                        # denominators replicated on psum rows 0:63 via the
                        # leading ones-columns of v; dims on rows 64:127.
                        ri = rip.tile([64, 512], F32, tag="ri", name=f"ri{p}_{g}")
                        nc.vector.reciprocal_approx_fast(ri[:], pv[0:64, :])
                        rows = slice(0, 64) if hh == 0 else slice(64, 128)
                        nc.vector.tensor_mul(outT[p][rows, cs], pv[64:128, :], ri[:])

                    chunks.append(first)
                    chunks.append(second)
                return chunks

            pending_pv = []
            for p in range(KT):
                qt, kt = qk_sb[p], qk_sb[KT + p]
                eAs, eBs = [], []
                for j in range(ST):
                    scA = ps_sc.tile([128, S], F32, tag="sc", name=f"scA{p}_{j}")
                    scB = ps_sc.tile([128, S], F32, tag="sc", name=f"scB{p}_{j}")
                    for h2 in range(2):
                        cs = slice(512 * h2, 512 * h2 + 512)
                        nc.tensor.matmul(
                            scA[:, cs],
                            kt[0:64, 128 * j : 128 * j + 128], qt[0:64, cs],
                            start=True, stop=True,
                        )
                        nc.tensor.matmul(
                            scB[:, cs],
                            kt[64:128, 128 * j : 128 * j + 128], qt[64:128, cs],
                            start=True, stop=True,
                        )
                    # exp split: head A exact on ACT; head B Schraudolph on
                    # DVE (bf16 bit-trick), except 2 of 8 j on ACT to balance
                    eA = xpa.tile([128, S], BF16, tag="expA", name=f"eA{p}_{j}")
                    nc.scalar.activation(eA[:], scA[:], AF.Exp, scale=0.125)
                    eB = xpb.tile([128, S], BF16, tag="expB", name=f"eB{p}_{j}")
                    if j in (3, 7):
                        nc.scalar.activation(eB[:], scB[:], AF.Exp, scale=0.125)
                    else:
                        nc.vector.tensor_scalar(eB[:].bitcast(I16), scB[:], SC_EXP, BC_EXP, OP.mult, OP.add)
                    eAs.append(eA)
                    eBs.append(eB)
                    if pending_pv:
                        pending_pv.pop(0)()
                pending_pv.extend(mk_pv_chunks(p, eAs, eBs))
            for c in pending_pv:
                c()

        # ---------------- stage 4: output projection ------------------------
        with ExitStack() as late:
            yp = late.enter_context(tc.tile_pool(name="yp", bufs=2))
            ps_y = late.enter_context(tc.tile_pool(name="psy", bufs=2, space="PSUM"))
            for mt in range(ST):
                ps = ps_y.tile([128, C], F32, tag="y")
                for cl, cw in ((0, 512), (512, 256)):
                    for k6 in range(KT):
                        nc.tensor.matmul(
                            ps[:, cl : cl + cw],
                            outT[k6][:, 128 * mt : 128 * mt + 128],
                            pwT[:, k6, cl : cl + cw],
                            start=(k6 == 0), stop=False,
                        )
                    nc.tensor.matmul(
                        ps[:, cl : cl + cw], ones_r[:], pbias[:, cl : cl + cw],
                        start=False, stop=True,
                    )
                y = yp.tile([128, C], F32, tag="y_sb")
                nc.scalar.activation(y[:], ps[:], AF.Identity, scale=1.0)
                nc.sync.dma_start(out_d[128 * mt : 128 * mt + 128, :], y[:])


LDW_OPT = False  # walrus LDW-opt rejects bass InstLdweights


def _patch_walrus():
    import concourse.bass_utils as _bu
    if getattr(_bu, "_ldwopt_patched", False):
        return
    _orig = _bu.run_command

    def _patched(cmd, **kw):
        if LDW_OPT and isinstance(cmd, list):
            cmd = ["--enable-ldw-opt=true" if c == "--enable-ldw-opt=false" else c for c in cmd]
        return _orig(cmd, **kw)

    _bu.run_command = _patched
    _bu._ldwopt_patched = True


def build():
    if "nc" in _CACHE:
        return _CACHE["nc"]
    _patch_walrus()
    nc = bacc.Bacc("TRN2", target_bir_lowering=False, debug=False)
    d = {}

    def din(name, shape, dt):
        d[name] = nc.dram_tensor(name, shape, dt, kind="ExternalInput").ap()

    din("xT", [C, S], BF16)
    din("wqkT", [C, 2 * C], BF16)
    din("wvT", [C, C], BF16)
    din("pwT", [C, C], BF16)
    din("bqk", [128, 2 * KT], F32)
    din("vbias", [1, C], BF16)
    din("pbias", [1, C], BF16)
    din("cosq", [128, S], BF16)
    din("sinq", [128, S], BF16)
    din("cosk", [128, S], BF16)
    din("sink", [128, S], BF16)
    din("ebig", [128, 22], BF16)
    din("epsc", [12, 1], F32)
    din("sel", [12, 6 * 128], BF16)
    out_d = nc.dram_tensor("out", [S, C], F32, kind="ExternalOutput").ap()

    with tile.TileContext(nc) as tc:
        _graph(tc, d, out_d)
    nc.compile()
    _CACHE["nc"] = nc
    return nc


def make_in_maps(x, qkv_w, qkv_b, q_norm_w, k_norm_w, proj_w, proj_b):
    shared = _prep_shared(qkv_w, qkv_b, q_norm_w, k_norm_w, proj_w, proj_b)
    x = np.asarray(x, np.float32)
    in_maps = []
    for b in range(NCORES):
        m = dict(shared)
        m["xT"] = np.ascontiguousarray(x[b].T).astype(NP_BF16)
        in_maps.append(m)
    return in_maps


def run(in_maps, trace=False, **kw):
    nc = build()
    return run_bass_kernel_spmd(nc, in_maps, core_ids=list(range(NCORES)), trace=trace, **kw)


def kernel(x, qkv_w, qkv_b, q_norm_w, k_norm_w, proj_w, proj_b):
    in_maps = make_in_maps(x, qkv_w, qkv_b, q_norm_w, k_norm_w, proj_w, proj_b)
    res = run(in_maps)
    return np.stack([np.asarray(res.results[i]["out"]) for i in range(NCORES)]).astype(np.float32)


if __name__ == "__main__":
    rng = np.random.default_rng(0)
    ins = {
        "x": rng.standard_normal((B, S, C)).astype(np.float32),
        "qkv_w": (rng.standard_normal((3 * C, C)) * C**-0.5).astype(np.float32),
        "qkv_b": (rng.standard_normal(3 * C) * 0.02).astype(np.float32),
        "q_norm_w": np.ones(D, np.float32),
        "k_norm_w": np.ones(D, np.float32),
        "proj_w": (rng.standard_normal((C, C)) * C**-0.5).astype(np.float32),
        "proj_b": (rng.standard_normal(C) * 0.02).astype(np.float32),
    }
    y = kernel(**ins)
    print("out", y.shape, y.dtype)


# revision 25
# speedup vs baseline: 1.2374x; 1.0553x over previous
"""Trainium2 Bass kernel for nn_Attention_26628797235884.

12-head attention block (qkv proj + per-head RMS norm + 2D RoPE + softmax
attention + output proj), batch 8 x seq 1024 x dim 768, data-parallel over
batch across 8 NeuronCores (batch b -> core b, weights replicated).

Per-core design (v2):
  - q,k computed feature-major ([feat, seq]) so QK^T and PV contract along
    partitions without transposes; v in natural [seq, feat] layout.
  - exp split across two engines: head-A exps on ACT (exact Exp), head-B
    exps on DVE via a Schraudolph fp16 bit-trick (x*S+B -> int16, bit-viewed
    as fp16); softmax averaging dilutes the ~1.8% rms weight error ~50x.
  - PV lhsT widened to M=128 with 64 replicated ones-columns (costs no PE
    time: matmul cost = N only) so the softmax denominator lands replicated
    on psum partitions 64:127 -> 64-lane DVE reciprocal + one mixed-base DVE
    multiply finish the epilogue (no DMA / gpsimd broadcast).
  - qkv/proj biases folded into the matmuls via K=1 ones-row passes; psum
    drains on ACT; sum-of-squares via ACT Square + indicator matmul.
  - DMA ordered so x/qkv weights land first; proj weights last.
All matmuls bf16/fp16 with fp32 accumulation.
"""

import sys

import numpy as np
import ml_dtypes

try:
    import concourse.bass as bass  # noqa: F401
except ImportError:  # pragma: no cover
    sys.path.insert(0, "/opt/trn_rl_repo")

import concourse.tile as tile
from concourse import bacc, mybir
from concourse.bass_utils import run_bass_kernel_spmd

BF16 = mybir.dt.bfloat16
F16 = mybir.dt.float16
I16 = mybir.dt.int16
F32 = mybir.dt.float32
AF = mybir.ActivationFunctionType
OP = mybir.AluOpType
NP_BF16 = ml_dtypes.bfloat16

B, S, C, H, D = 8, 1024, 768, 12, 64
KT = C // 128          # 6 contraction tiles over the model dim
ST = S // 128          # 8 seq tiles
NCORES = 8
EPS = 1e-6
EPS_V = 1e-6
PAIRSWAP32 = [i ^ 1 for i in range(32)]

# bf16 Schraudolph exp: bits16 = trunc(x*0.125*128*log2e + (127*128+0.5-C))
# (bf16 not fp16: fp16 matmuls stream at half the bf16 rate on TRN2 HW)
SC_EXP = 128.0 * 1.4426950408889634 * 0.125
BC_EXP = 127.0 * 128.0 + 0.5 - 7.40

_CACHE = {}


# --------------------------------------------------------------------------
# host-side constant prep
# --------------------------------------------------------------------------

def _rope_tables():
    ROPE_DIM, PT_SEQ, FT_SEQ, THETA = 32, 16, 32, 10000.0
    freqs = 1.0 / (THETA ** (np.arange(0, ROPE_DIM, 2, dtype=np.float32)[: ROPE_DIM // 2] / ROPE_DIM))
    t = np.arange(FT_SEQ, dtype=np.float32) / FT_SEQ * PT_SEQ
    f = np.einsum("i,j->ij", t, freqs)
    f = np.repeat(f, 2, axis=-1)
    fh = np.broadcast_to(f[:, None, :], (FT_SEQ, FT_SEQ, ROPE_DIM))
    fw = np.broadcast_to(f[None, :, :], (FT_SEQ, FT_SEQ, ROPE_DIM))
    f2 = np.concatenate([fh, fw], axis=-1).reshape(FT_SEQ * FT_SEQ, 2 * ROPE_DIM)
    return np.cos(f2).astype(np.float32), np.sin(f2).astype(np.float32)


def _prep_shared(qkv_w, qkv_b, q_norm_w, k_norm_w, proj_w, proj_b):
    f32 = np.float32
    cos, sin = _rope_tables()                 # [S, D]
    pair = np.arange(D) ^ 1
    sa = sin.copy()
    sa[:, 0::2] *= -1.0                       # sign-folded sin for rotate_half

    def mk(tab, w):                           # -> [128, S] bf16, 2 heads stacked
        t = (tab * w[None, :]).T.astype(f32)  # [64, S]
        return np.ascontiguousarray(np.vstack([t, t])).astype(NP_BF16)

    qw = np.asarray(q_norm_w, f32)
    kw = np.asarray(k_norm_w, f32)
    shared = {
        "wqkT": np.ascontiguousarray(np.asarray(qkv_w, f32)[: 2 * C].T).astype(NP_BF16),
        "wvT": np.ascontiguousarray(np.asarray(qkv_w, f32)[2 * C :].T).astype(NP_BF16),
        "pwT": np.ascontiguousarray(np.asarray(proj_w, f32).T).astype(NP_BF16),
        "cosq": mk(cos, qw),
        "sinq": mk(sa, qw[pair]),
        "cosk": mk(cos, kw),
        "sink": mk(sa, kw[pair]),
    }
    b = np.asarray(qkv_b, f32)
    shared["bqk"] = np.ascontiguousarray(b[: 2 * C].reshape(2 * KT, 128).T)  # [128, 12]
    shared["vbias"] = np.ascontiguousarray(b[2 * C :][None, :]).astype(NP_BF16)       # [1, 768]
    shared["pbias"] = np.ascontiguousarray(np.asarray(proj_b, f32)[None, :]).astype(NP_BF16)
    # E_big[:, 10 - 2i : 22 - 2i] is a [128, 12] indicator lhsT whose column
    # 2i+g selects partition half g — lets 6 m-tiles' group-sums accumulate
    # into one [12, S] PSUM tensor.
    ebig = np.zeros((128, 22), NP_BF16)
    ebig[0:64, 10] = 1
    ebig[64:128, 11] = 1
    shared["ebig"] = ebig
    # sel[:, 128i:128i+128] broadcasts rinv rows (2i, 2i+1) to the 2 head
    # halves of a [128, S] field via lhsT.T @ rinv.
    sel = np.zeros((12, 6 * 128), NP_BF16)
    for i in range(6):
        sel[2 * i, 128 * i : 128 * i + 64] = 1
        sel[2 * i + 1, 128 * i + 64 : 128 * i + 128] = 1
    shared["sel"] = sel
    shared["epsc"] = np.full((12, 1), EPS_V, dtype=np.float32)
    return shared


# --------------------------------------------------------------------------
# device graph
# --------------------------------------------------------------------------

def _graph(tc, d, out_d):
    nc = tc.nc
    from contextlib import ExitStack

    with ExitStack() as big:
        main = big.enter_context(tc.tile_pool(name="main", bufs=1))

        qk_sb = [main.tile([128, S], BF16, tag=f"qk{m}", name=f"qk{m}") for m in range(2 * KT)]
        v_sb = [main.tile([128, H * 128], BF16, tag=f"v{j}", name=f"v{j}") for j in range(ST)]
        outT = [main.tile([128, S], BF16, tag=f"ot{p}", name=f"ot{p}") for p in range(KT)]
        ones_r = main.tile([1, 128], BF16, tag="ones_r")
        pwT = main.tile([128, KT, C], BF16, tag="pwT")
        pbias = main.tile([1, C], BF16, tag="pbias")

        # ---------------- stage 1+2: projections, norm, rope ----------------
        with ExitStack() as early:
            ep = early.enter_context(tc.tile_pool(name="early", bufs=1))
            w1 = early.enter_context(tc.tile_pool(name="w1", bufs=2))
            w1b = early.enter_context(tc.tile_pool(name="w1b", bufs=3))
            t1p = early.enter_context(tc.tile_pool(name="t1p", bufs=2))
            qkv_ps = early.enter_context(ExitStack())
            ps_mm = qkv_ps.enter_context(tc.tile_pool(name="psmm", bufs=3, space="PSUM"))
            ps_sq = qkv_ps.enter_context(tc.tile_pool(name="pssq", bufs=1, space="PSUM"))
            ps_fld = qkv_ps.enter_context(tc.tile_pool(name="psfld", bufs=1, space="PSUM"))

            # DMA order = need order: x + qkv weights, tables, v weights,
            # proj weights last. Coarse chunks: descriptor generation costs
            # ~0.6us of queue time per dma_start, so fewer is faster.
            xT = ep.tile([128, KT, S], BF16, tag="xT")
            xT_r = d["xT"].rearrange("(k p) s -> p k s", p=128)
            wqkT = ep.tile([128, KT, 2 * C], BF16, tag="wqkT")
            wqkT_r = d["wqkT"].rearrange("(k p) o -> p k o", p=128)
            wvT = ep.tile([128, KT, C], BF16, tag="wvT")
            wvT_r = d["wvT"].rearrange("(k p) o -> p k o", p=128)
            tabs = {}
            for nm in ("cosq", "sinq", "cosk", "sink"):
                tabs[nm] = ep.tile([128, S], BF16, tag=nm, name=nm)
            bqk = ep.tile([128, 2 * KT], F32, tag="bqk")
            engs = [nc.sync, nc.scalar, nc.gpsimd]
            for k in range(KT):
                engs[k % 3].dma_start(xT[:, k], xT_r[:, k])
            for k in range(KT):
                engs[k % 3].dma_start(wqkT[:, k, 0:768], wqkT_r[:, k, 0:768])
            for k in range(KT):
                engs[k % 3].dma_start(wqkT[:, k, 768:1536], wqkT_r[:, k, 768:1536])
            nc.sync.dma_start(bqk[:], d["bqk"][:])
            for nm in ("cosq", "sinq", "cosk", "sink"):
                nc.sync.dma_start(tabs[nm][:], d[nm][:])
            ebig = ep.tile([128, 22], BF16, tag="ebig")
            nc.sync.dma_start(ebig[:], d["ebig"][:])
            sel = ep.tile([12, 6 * 128], BF16, tag="sel")
            nc.sync.dma_start(sel[:], d["sel"][:])
            epsc = ep.tile([12, 1], F32, tag="epsc")
            nc.sync.dma_start(epsc[:], d["epsc"][:])
            for k in range(KT):
                nc.sync.dma_start(wvT[:, k], wvT_r[:, k])
            vbias = ep.tile([1, C], BF16, tag="vbias")
            nc.sync.dma_start(vbias[:], d["vbias"][:])
            pwT_r = d["pwT"].rearrange("(k p) o -> p k o", p=128)
            for k in range(KT):
                nc.sync.dma_start(pwT[:, k], pwT_r[:, k])
            nc.sync.dma_start(pbias[:], d["pbias"][:])

            nc.gpsimd.memset(ones_r[:], 1.0)
            for j in range(ST):
                # ones-columns 0:64 of every head: denominator replication.
                # Ones FIRST so Z lands on psum partitions 0:63 —
                # reciprocal_approx_fast at partition base 64 no-ops on HW.
                nc.gpsimd.memset(
                    v_sb[j][:].rearrange("p (h e) -> p h e", e=128)[:, :, 0:64], 1.0
                )

            batches = [[0, 1, 2, 6, 7, 8], [3, 4, 5, 9, 10, 11]]
            pending_tails = []
            for batch, ms in enumerate(batches):
                nb = len(ms)
                sqb = ps_sq.tile([2 * nb, S], F32, tag="sq", name=f"sqb{batch}")
                t1s = []
                for i, m in enumerate(ms):
                    if batch >= 1 and i == 2 and pending_tails:
                        pending_tails.pop(0)()  # previous batch's fields
                    ctab = tabs["cosq"] if m < KT else tabs["cosk"]
                    stab = tabs["sinq"] if m < KT else tabs["sink"]
                    t1 = t1p.tile([128, S], BF16, tag=f"t1_{i}", name=f"t1_{batch}_{i}")
                    t1s.append(t1)
                    t = w1b.tile([128, S], BF16, tag="t")
                    t2 = w1b.tile([128, S], BF16, tag="t2")
                    for h2 in range(2):
                        cs = slice(512 * h2, 512 * h2 + 512)
                        ps = ps_mm.tile([128, 512], F32, tag="mm", name=f"mm{batch}_{i}_{h2}")
                        for k in range(KT):
                            nc.tensor.matmul(
                                ps[:],
                                wqkT[:, k, 128 * m : 128 * m + 128],
                                xT[:, k, cs],
                                start=(k == 0),
                                stop=(k == KT - 1),
                            )
                        # two ACT readers of the psum: t = ps + b, t2 = (ps + b)^2
                        nc.scalar.activation(t[:, cs], ps[:], AF.Identity, bias=bqk[:, m : m + 1], scale=1.0)
                        nc.scalar.activation(t2[:, cs], ps[:], AF.Square, bias=bqk[:, m : m + 1], scale=1.0)
                        nc.tensor.matmul(
                            sqb[:, cs],
                            ebig[:, 10 - 2 * i : 10 - 2 * i + 2 * nb],
                            t2[:, cs],
                            start=(i == 0), stop=(i == nb - 1),
                        )
                    # rope over the full m-tile: u = t*cos ; vv = shuffle(t)*sinA
                    u = w1b.tile([128, S], BF16, tag="u")
                    nc.vector.tensor_mul(u[:], t[:], ctab[:])
                    tsh = w1b.tile([128, S], BF16, tag="tsh")
                    nc.vector.stream_shuffle(tsh[:], t[:], PAIRSWAP32)
                    vv = w1b.tile([128, S], BF16, tag="vv")
                    nc.vector.tensor_mul(vv[:], tsh[:], stab[:])
                    nc.gpsimd.tensor_add(t1[:], u[:], vv[:])
                rms = w1.tile([2 * nb, S], F32, tag="rms", name=f"rms{batch}")
                nc.scalar.activation(rms[:], sqb[:], AF.Sqrt, bias=epsc[:], scale=1.0 / D)
                rinv = w1.tile([2 * nb, S], F32, tag="rinv", name=f"rinv{batch}")
                nc.vector.reciprocal_approx_fast(rinv[:], rms[:])
                rinv_bf = w1.tile([2 * nb, S], BF16, tag="rinv_bf", name=f"rinvbf{batch}")
                nc.vector.tensor_copy(rinv_bf[:], rinv[:])

                def _mk_tail(ms=ms, t1s=t1s, rinv_bf=rinv_bf, nb=nb):
                    def _tail():
                        for i, m in enumerate(ms):
                            fldp = ps_fld.tile([128, S], F32, tag="fld", name=f"fld{m}")
                            for h2 in range(2):
                                cs = slice(512 * h2, 512 * h2 + 512)
                                nc.tensor.matmul(
                                    fldp[:, cs],
                                    sel[0 : 2 * nb, 128 * i : 128 * i + 128],
                                    rinv_bf[:, cs],
                                    start=True, stop=True,
                                )
                            nc.vector.tensor_mul(qk_sb[m][:], t1s[i][:], fldp[:])
                    return _tail
                pending_tails.append(_mk_tail())

            for t in pending_tails:
                t()
            pending_tails = []

            # ------------- stage 2: V projection (own psum scope) -----------
            # Dense PE work here overlaps the last batch's DVE/Pool tails.
            qkv_ps.close()
            ps_v = early.enter_context(tc.tile_pool(name="psv", bufs=2, space="PSUM"))
            for j in range(ST):
                vview = v_sb[j][:].rearrange("p (h e) -> p h e", e=128)
                for lo, cw, nh in ((0, 512, 8), (512, 256, 4)):
                    psv = ps_v.tile([128, cw], F32, tag=f"vmm{cw}", name=f"vmm{j}_{lo}")
                    for k in range(KT):
                        nc.tensor.matmul(
                            psv[:], xT[:, k, 128 * j : 128 * j + 128],
                            wvT[:, k, lo : lo + cw],
                            start=(k == 0), stop=False,
                        )
                    nc.tensor.matmul(
                        psv[:], ones_r[:], vbias[:, lo : lo + cw],
                        start=False, stop=True,
                    )
                    nc.scalar.activation(
                        vview[:, lo // 64 : lo // 64 + nh, 64:128], psv[:], AF.Identity, scale=1.0
                    )

        # ---------------- stage 3: attention, software-pipelined ------------
        # PV is organized as per-(head, q-half) accumulation groups over all
        # j (no exp dependencies once the pair's exps exist) and interleaved
        # between the NEXT pair's score matmuls, so the PE has dense filler
        # work instead of chain-waiting on exp slot recycling.
        with ExitStack() as att:
            xpa = att.enter_context(tc.tile_pool(name="attxa", bufs=12))
            xpb = att.enter_context(tc.tile_pool(name="attxb", bufs=12))
            rip = att.enter_context(tc.tile_pool(name="attri", bufs=3))
            ps_sc = att.enter_context(tc.tile_pool(name="pssc", bufs=3, space="PSUM"))
            ps_pv = att.enter_context(tc.tile_pool(name="pspv", bufs=2, space="PSUM"))

            def mk_pv_chunks(p, eAs, eBs):
                """8 closures: two per (head, q-half) group; the second also
                emits the group's recip+normalize epilogue."""
                chunks = []
                for g in range(4):
                    hh, qh = g // 2, g % 2
                    cs = slice(512 * qh, 512 * qh + 512)
                    es = eAs if hh == 0 else eBs
                    h = 2 * p + hh
                    box = {}

                    def first(box=box, h=h, cs=cs, es=es, p=p, g=g):
                        pv = ps_pv.tile([128, 512], F32, tag="pv", name=f"pv{p}_{g}")
                        box["pv"] = pv
                        for j in range(4):
                            vva = v_sb[j][:].rearrange("p (h e) -> p h e", e=128)
                            nc.tensor.matmul(pv[:], vva[:, h, :], es[j][:, cs],
                                             start=(j == 0), stop=False)

                    def second(box=box, h=h, cs=cs, es=es, hh=hh, p=p, g=g):
                        pv = box["pv"]
                        for j in range(4, ST):
                            vva = v_sb[j][:].rearrange("p (h e) -> p h e", e=128)
                            nc.tensor.matmul(pv[:], vva[:, h, :], es[j][:, cs],
                                             start=False, stop=(j == ST - 1))
                        # denominators replicated on psum rows 0:63 via the
                        # leading ones-columns of v; dims on rows 64:127.
                        ri = rip.tile([64, 512], F32, tag="ri", name=f"ri{p}_{g}")
                        nc.vector.reciprocal_approx_fast(ri[:], pv[0:64, :])
                        rows = slice(0, 64) if hh == 0 else slice(64, 128)
                        nc.vector.tensor_mul(outT[p][rows, cs], pv[64:128, :], ri[:])

                    chunks.append(first)
                    chunks.append(second)
                return chunks

            pending_pv = []
            for p in range(KT):
                qt, kt = qk_sb[p], qk_sb[KT + p]
                eAs, eBs = [], []
                for j in range(ST):
                    scA = ps_sc.tile([128, S], F32, tag="sc", name=f"scA{p}_{j}")
                    scB = ps_sc.tile([128, S], F32, tag="sc", name=f"scB{p}_{j}")
                    for h2 in range(2):
                        cs = slice(512 * h2, 512 * h2 + 512)
                        nc.tensor.matmul(
                            scA[:, cs],
                            kt[0:64, 128 * j : 128 * j + 128], qt[0:64, cs],
                            start=True, stop=True,
                        )
                        nc.tensor.matmul(
                            scB[:, cs],
                            kt[64:128, 128 * j : 128 * j + 128], qt[64:128, cs],
                            start=True, stop=True,
                        )
                    # exp split: head A exact on ACT; head B Schraudolph on
                    # DVE (bf16 bit-trick), except 2 of 8 j on ACT to balance
                    eA = xpa.tile([128, S], BF16, tag="expA", name=f"eA{p}_{j}")
                    nc.scalar.activation(eA[:], scA[:], AF.Exp, scale=0.125)
                    eB = xpb.tile([128, S], BF16, tag="expB", name=f"eB{p}_{j}")
                    if j in (3, 7):
                        nc.scalar.activation(eB[:], scB[:], AF.Exp, scale=0.125)
                    else:
                        nc.vector.tensor_scalar(eB[:].bitcast(I16), scB[:], SC_EXP, BC_EXP, OP.mult, OP.add)
                    eAs.append(eA)
                    eBs.append(eB)
                    if pending_pv:
                        pending_pv.pop(0)()
                pending_pv.extend(mk_pv_chunks(p, eAs, eBs))
            for c in pending_pv:
                c()

        # ---------------- stage 4: output projection ------------------------
        with ExitStack() as late:
            yp = late.enter_context(tc.tile_pool(name="yp", bufs=2))
            ps_y = late.enter_context(tc.tile_pool(name="psy", bufs=2, space="PSUM"))
            for mt in range(ST):
                ps = ps_y.tile([128, C], F32, tag="y")
                for cl, cw in ((0, 512), (512, 256)):
                    for k6 in range(KT):
                        nc.tensor.matmul(
                            ps[:, cl : cl + cw],
                            outT[k6][:, 128 * mt : 128 * mt + 128],
                            pwT[:, k6, cl : cl + cw],
                            start=(k6 == 0), stop=False,
                        )
                    nc.tensor.matmul(
                        ps[:, cl : cl + cw], ones_r[:], pbias[:, cl : cl + cw],
                        start=False, stop=True,
                    )
                y = yp.tile([128, C], F32, tag="y_sb")
                nc.scalar.activation(y[:], ps[:], AF.Identity, scale=1.0)
                nc.sync.dma_start(out_d[128 * mt : 128 * mt + 128, :], y[:])


LDW_OPT = False  # walrus LDW-opt rejects bass InstLdweights


def _patch_walrus():
    import concourse.bass_utils as _bu
    if getattr(_bu, "_ldwopt_patched", False):
        return
    _orig = _bu.run_command

    def _patched(cmd, **kw):
        if LDW_OPT and isinstance(cmd, list):
            cmd = ["--enable-ldw-opt=true" if c == "--enable-ldw-opt=false" else c for c in cmd]
        return _orig(cmd, **kw)

    _bu.run_command = _patched
    _bu._ldwopt_patched = True


def build():
    if "nc" in _CACHE:
        return _CACHE["nc"]
    _patch_walrus()
    nc = bacc.Bacc("TRN2", target_bir_lowering=False, debug=False)
    d = {}

    def din(name, shape, dt):
        d[name] = nc.dram_tensor(name, shape, dt, kind="ExternalInput").ap()

    din("xT", [C, S], BF16)
    din("wqkT", [C, 2 * C], BF16)
    din("wvT", [C, C], BF16)
    din("pwT", [C, C], BF16)
    din("bqk", [128, 2 * KT], F32)
    din("vbias", [1, C], BF16)
    din("pbias", [1, C], BF16)
    din("cosq", [128, S], BF16)
    din("sinq", [128, S], BF16)
    din("cosk", [128, S], BF16)
    din("sink", [128, S], BF16)
    din("ebig", [128, 22], BF16)
    din("epsc", [12, 1], F32)
    din("sel", [12, 6 * 128], BF16)
    out_d = nc.dram_tensor("out", [S, C], F32, kind="ExternalOutput").ap()

    with tile.TileContext(nc) as tc:
        _graph(tc, d, out_d)
    nc.compile()
    _CACHE["nc"] = nc
    return nc


def make_in_maps(x, qkv_w, qkv_b, q_norm_w, k_norm_w, proj_w, proj_b):
    shared = _prep_shared(qkv_w, qkv_b, q_norm_w, k_norm_w, proj_w, proj_b)
    x = np.asarray(x, np.float32)
    in_maps = []
    for b in range(NCORES):
        m = dict(shared)
        m["xT"] = np.ascontiguousarray(x[b].T).astype(NP_BF16)
        in_maps.append(m)
    return in_maps


def run(in_maps, trace=False, **kw):
    nc = build()
    return run_bass_kernel_spmd(nc, in_maps, core_ids=list(range(NCORES)), trace=trace, **kw)


def kernel(x, qkv_w, qkv_b, q_norm_w, k_norm_w, proj_w, proj_b):
    in_maps = make_in_maps(x, qkv_w, qkv_b, q_norm_w, k_norm_w, proj_w, proj_b)
    res = run(in_maps)
    return np.stack([np.asarray(res.results[i]["out"]) for i in range(NCORES)]).astype(np.float32)


if __name__ == "__main__":
    rng = np.random.default_rng(0)
    ins = {
        "x": rng.standard_normal((B, S, C)).astype(np.float32),
        "qkv_w": (rng.standard_normal((3 * C, C)) * C**-0.5).astype(np.float32),
        "qkv_b": (rng.standard_normal(3 * C) * 0.02).astype(np.float32),
        "q_norm_w": np.ones(D, np.float32),
        "k_norm_w": np.ones(D, np.float32),
        "proj_w": (rng.standard_normal((C, C)) * C**-0.5).astype(np.float32),
        "proj_b": (rng.standard_normal(C) * 0.02).astype(np.float32),
    }
    y = kernel(**ins)
    print("out", y.shape, y.dtype)


# revision 26
# speedup vs baseline: 1.2475x; 1.0081x over previous
"""Trainium2 Bass kernel for nn_Attention_26628797235884.

12-head attention block (qkv proj + per-head RMS norm + 2D RoPE + softmax
attention + output proj), batch 8 x seq 1024 x dim 768, data-parallel over
batch across 8 NeuronCores (batch b -> core b, weights replicated).

Per-core design (v2):
  - q,k computed feature-major ([feat, seq]) so QK^T and PV contract along
    partitions without transposes; v in natural [seq, feat] layout.
  - exp split across two engines: head-A exps on ACT (exact Exp), head-B
    exps on DVE via a Schraudolph fp16 bit-trick (x*S+B -> int16, bit-viewed
    as fp16); softmax averaging dilutes the ~1.8% rms weight error ~50x.
  - PV lhsT widened to M=128 with 64 replicated ones-columns (costs no PE
    time: matmul cost = N only) so the softmax denominator lands replicated
    on psum partitions 64:127 -> 64-lane DVE reciprocal + one mixed-base DVE
    multiply finish the epilogue (no DMA / gpsimd broadcast).
  - qkv/proj biases folded into the matmuls via K=1 ones-row passes; psum
    drains on ACT; sum-of-squares via ACT Square + indicator matmul.
  - DMA ordered so x/qkv weights land first; proj weights last.
All matmuls bf16/fp16 with fp32 accumulation.
"""

import sys

import numpy as np
import ml_dtypes

try:
    import concourse.bass as bass  # noqa: F401
except ImportError:  # pragma: no cover
    sys.path.insert(0, "/opt/trn_rl_repo")

import concourse.tile as tile
from concourse import bacc, mybir
from concourse.bass_utils import run_bass_kernel_spmd

BF16 = mybir.dt.bfloat16
F16 = mybir.dt.float16
I16 = mybir.dt.int16
F32 = mybir.dt.float32
AF = mybir.ActivationFunctionType
OP = mybir.AluOpType
NP_BF16 = ml_dtypes.bfloat16

B, S, C, H, D = 8, 1024, 768, 12, 64
KT = C // 128          # 6 contraction tiles over the model dim
ST = S // 128          # 8 seq tiles
NCORES = 8
EPS = 1e-6
EPS_V = 1e-6
PAIRSWAP32 = [i ^ 1 for i in range(32)]

# bf16 Schraudolph exp: bits16 = trunc(x*0.125*128*log2e + (127*128+0.5-C))
# (bf16 not fp16: fp16 matmuls stream at half the bf16 rate on TRN2 HW)
SC_EXP = 128.0 * 1.4426950408889634 * 0.125
BC_EXP = 127.0 * 128.0 + 0.5 - 7.40

_CACHE = {}


# --------------------------------------------------------------------------
# host-side constant prep
# --------------------------------------------------------------------------

def _rope_tables():
    ROPE_DIM, PT_SEQ, FT_SEQ, THETA = 32, 16, 32, 10000.0
    freqs = 1.0 / (THETA ** (np.arange(0, ROPE_DIM, 2, dtype=np.float32)[: ROPE_DIM // 2] / ROPE_DIM))
    t = np.arange(FT_SEQ, dtype=np.float32) / FT_SEQ * PT_SEQ
    f = np.einsum("i,j->ij", t, freqs)
    f = np.repeat(f, 2, axis=-1)
    fh = np.broadcast_to(f[:, None, :], (FT_SEQ, FT_SEQ, ROPE_DIM))
    fw = np.broadcast_to(f[None, :, :], (FT_SEQ, FT_SEQ, ROPE_DIM))
    f2 = np.concatenate([fh, fw], axis=-1).reshape(FT_SEQ * FT_SEQ, 2 * ROPE_DIM)
    return np.cos(f2).astype(np.float32), np.sin(f2).astype(np.float32)


def _prep_shared(qkv_w, qkv_b, q_norm_w, k_norm_w, proj_w, proj_b):
    f32 = np.float32
    cos, sin = _rope_tables()                 # [S, D]
    pair = np.arange(D) ^ 1
    sa = sin.copy()
    sa[:, 0::2] *= -1.0                       # sign-folded sin for rotate_half

    def mk(tab, w):                           # -> [128, S] bf16, 2 heads stacked
        t = (tab * w[None, :]).T.astype(f32)  # [64, S]
        return np.ascontiguousarray(np.vstack([t, t])).astype(NP_BF16)

    qw = np.asarray(q_norm_w, f32)
    kw = np.asarray(k_norm_w, f32)
    shared = {
        "wqkT": np.ascontiguousarray(np.asarray(qkv_w, f32)[: 2 * C].T).astype(NP_BF16),
        "wvT": np.ascontiguousarray(np.asarray(qkv_w, f32)[2 * C :].T).astype(NP_BF16),
        "pwT": np.ascontiguousarray(np.asarray(proj_w, f32).T).astype(NP_BF16),
        "cosq": mk(cos, qw),
        "sinq": mk(sa, qw[pair]),
        "cosk": mk(cos, kw),
        "sink": mk(sa, kw[pair]),
    }
    b = np.asarray(qkv_b, f32)
    shared["bqk"] = np.ascontiguousarray(b[: 2 * C].reshape(2 * KT, 128).T)  # [128, 12]
    shared["vbias"] = np.ascontiguousarray(b[2 * C :][None, :]).astype(NP_BF16)       # [1, 768]
    shared["pbias"] = np.ascontiguousarray(np.asarray(proj_b, f32)[None, :]).astype(NP_BF16)
    # E_big[:, 10 - 2i : 22 - 2i] is a [128, 12] indicator lhsT whose column
    # 2i+g selects partition half g — lets 6 m-tiles' group-sums accumulate
    # into one [12, S] PSUM tensor.
    ebig = np.zeros((128, 22), NP_BF16)
    ebig[0:64, 10] = 1
    ebig[64:128, 11] = 1
    shared["ebig"] = ebig
    # sel[:, 128i:128i+128] broadcasts rinv rows (2i, 2i+1) to the 2 head
    # halves of a [128, S] field via lhsT.T @ rinv.
    sel = np.zeros((12, 6 * 128), NP_BF16)
    for i in range(6):
        sel[2 * i, 128 * i : 128 * i + 64] = 1
        sel[2 * i + 1, 128 * i + 64 : 128 * i + 128] = 1
    shared["sel"] = sel
    shared["epsc"] = np.full((12, 1), EPS_V, dtype=np.float32)
    return shared


# --------------------------------------------------------------------------
# device graph
# --------------------------------------------------------------------------

def _graph(tc, d, out_d):
    nc = tc.nc
    from contextlib import ExitStack

    with ExitStack() as big:
        main = big.enter_context(tc.tile_pool(name="main", bufs=1))

        qk_sb = [main.tile([128, S], BF16, tag=f"qk{m}", name=f"qk{m}") for m in range(2 * KT)]
        v_sb = [main.tile([128, H * 128], BF16, tag=f"v{j}", name=f"v{j}") for j in range(ST)]
        outT = [main.tile([128, S], BF16, tag=f"ot{p}", name=f"ot{p}") for p in range(KT)]
        ones_r = main.tile([1, 128], BF16, tag="ones_r")
        pwT = main.tile([128, KT, C], BF16, tag="pwT")
        pbias = main.tile([1, C], BF16, tag="pbias")

        # ---------------- stage 1+2: projections, norm, rope ----------------
        with ExitStack() as early:
            ep = early.enter_context(tc.tile_pool(name="early", bufs=1))
            w1 = early.enter_context(tc.tile_pool(name="w1", bufs=2))
            w1b = early.enter_context(tc.tile_pool(name="w1b", bufs=3))
            t1p = early.enter_context(tc.tile_pool(name="t1p", bufs=2))
            qkv_ps = early.enter_context(ExitStack())
            ps_mm = qkv_ps.enter_context(tc.tile_pool(name="psmm", bufs=4, space="PSUM"))
            ps_sq = qkv_ps.enter_context(tc.tile_pool(name="pssq", bufs=1, space="PSUM"))
            ps_fld = qkv_ps.enter_context(tc.tile_pool(name="psfld", bufs=1, space="PSUM"))

            # DMA order = need order: x + qkv weights, tables, v weights,
            # proj weights last. Coarse chunks: descriptor generation costs
            # ~0.6us of queue time per dma_start, so fewer is faster.
            xT = ep.tile([128, KT, S], BF16, tag="xT")
            xT_r = d["xT"].rearrange("(k p) s -> p k s", p=128)
            wqkT = ep.tile([128, KT, 2 * C], BF16, tag="wqkT")
            wqkT_r = d["wqkT"].rearrange("(k p) o -> p k o", p=128)
            wvT = ep.tile([128, KT, C], BF16, tag="wvT")
            wvT_r = d["wvT"].rearrange("(k p) o -> p k o", p=128)
            tabs = {}
            for nm in ("cosq", "sinq", "cosk", "sink"):
                tabs[nm] = ep.tile([128, S], BF16, tag=nm, name=nm)
            bqk = ep.tile([128, 2 * KT], F32, tag="bqk")
            for k in range(KT):
                nc.sync.dma_start(xT[:, k], xT_r[:, k])
                nc.scalar.dma_start(wqkT[:, k, 0:1024], wqkT_r[:, k, 0:1024])
                nc.gpsimd.dma_start(wqkT[:, k, 1024:1536], wqkT_r[:, k, 1024:1536])
            nc.sync.dma_start(bqk[:], d["bqk"][:])
            for nm in ("cosq", "sinq", "cosk", "sink"):
                nc.sync.dma_start(tabs[nm][:], d[nm][:])
            ebig = ep.tile([128, 22], BF16, tag="ebig")
            nc.sync.dma_start(ebig[:], d["ebig"][:])
            sel = ep.tile([12, 6 * 128], BF16, tag="sel")
            nc.sync.dma_start(sel[:], d["sel"][:])
            epsc = ep.tile([12, 1], F32, tag="epsc")
            nc.sync.dma_start(epsc[:], d["epsc"][:])
            for k in range(KT):
                nc.sync.dma_start(wvT[:, k], wvT_r[:, k])
            vbias = ep.tile([1, C], BF16, tag="vbias")
            nc.sync.dma_start(vbias[:], d["vbias"][:])
            pwT_r = d["pwT"].rearrange("(k p) o -> p k o", p=128)
            for k in range(KT):
                nc.sync.dma_start(pwT[:, k], pwT_r[:, k])
            nc.sync.dma_start(pbias[:], d["pbias"][:])

            nc.gpsimd.memset(ones_r[:], 1.0)
            for j in range(ST):
                # ones-columns 0:64 of every head: denominator replication.
                # Ones FIRST so Z lands on psum partitions 0:63 —
                # reciprocal_approx_fast at partition base 64 no-ops on HW.
                nc.gpsimd.memset(
                    v_sb[j][:].rearrange("p (h e) -> p h e", e=128)[:, :, 0:64], 1.0
                )

            batches = [[0, 1, 2, 6, 7, 8], [3, 4, 5, 9, 10, 11]]
            pending_tails = []
            for batch, ms in enumerate(batches):
                nb = len(ms)
                sqb = ps_sq.tile([2 * nb, S], F32, tag="sq", name=f"sqb{batch}")
                t1s = []
                for i, m in enumerate(ms):
                    if batch >= 1 and i == 2 and pending_tails:
                        pending_tails.pop(0)()  # previous batch's fields
                    ctab = tabs["cosq"] if m < KT else tabs["cosk"]
                    stab = tabs["sinq"] if m < KT else tabs["sink"]
                    t1 = t1p.tile([128, S], BF16, tag=f"t1_{i}", name=f"t1_{batch}_{i}")
                    t1s.append(t1)
                    t = w1b.tile([128, S], BF16, tag="t")
                    t2 = w1b.tile([128, S], BF16, tag="t2")
                    for h2 in range(2):
                        cs = slice(512 * h2, 512 * h2 + 512)
                        ps = ps_mm.tile([128, 512], F32, tag="mm", name=f"mm{batch}_{i}_{h2}")
                        for k in range(KT):
                            nc.tensor.matmul(
                                ps[:],
                                wqkT[:, k, 128 * m : 128 * m + 128],
                                xT[:, k, cs],
                                start=(k == 0),
                                stop=(k == KT - 1),
                            )
                        # two ACT readers of the psum: t = ps + b, t2 = (ps + b)^2
                        nc.scalar.activation(t[:, cs], ps[:], AF.Identity, bias=bqk[:, m : m + 1], scale=1.0)
                        nc.scalar.activation(t2[:, cs], ps[:], AF.Square, bias=bqk[:, m : m + 1], scale=1.0)
                        nc.tensor.matmul(
                            sqb[:, cs],
                            ebig[:, 10 - 2 * i : 10 - 2 * i + 2 * nb],
                            t2[:, cs],
                            start=(i == 0), stop=(i == nb - 1),
                        )
                    # rope over the full m-tile: u = t*cos ; vv = shuffle(t)*sinA
                    u = w1b.tile([128, S], BF16, tag="u")
                    nc.vector.tensor_mul(u[:], t[:], ctab[:])
                    tsh = w1b.tile([128, S], BF16, tag="tsh")
                    nc.vector.stream_shuffle(tsh[:], t[:], PAIRSWAP32)
                    vv = w1b.tile([128, S], BF16, tag="vv")
                    nc.vector.tensor_mul(vv[:], tsh[:], stab[:])
                    nc.gpsimd.tensor_add(t1[:], u[:], vv[:])
                rms = w1.tile([2 * nb, S], F32, tag="rms", name=f"rms{batch}")
                nc.scalar.activation(rms[:], sqb[:], AF.Sqrt, bias=epsc[:], scale=1.0 / D)
                rinv = w1.tile([2 * nb, S], F32, tag="rinv", name=f"rinv{batch}")
                nc.vector.reciprocal_approx_fast(rinv[:], rms[:])
                rinv_bf = w1.tile([2 * nb, S], BF16, tag="rinv_bf", name=f"rinvbf{batch}")
                nc.vector.tensor_copy(rinv_bf[:], rinv[:])

                def _mk_tail(ms=ms, t1s=t1s, rinv_bf=rinv_bf, nb=nb):
                    def _tail():
                        for i, m in enumerate(ms):
                            fldp = ps_fld.tile([128, S], F32, tag="fld", name=f"fld{m}")
                            for h2 in range(2):
                                cs = slice(512 * h2, 512 * h2 + 512)
                                nc.tensor.matmul(
                                    fldp[:, cs],
                                    sel[0 : 2 * nb, 128 * i : 128 * i + 128],
                                    rinv_bf[:, cs],
                                    start=True, stop=True,
                                )
                            nc.vector.tensor_mul(qk_sb[m][:], t1s[i][:], fldp[:])
                    return _tail
                pending_tails.append(_mk_tail())

            for t in pending_tails:
                t()
            pending_tails = []

            # ------------- stage 2: V projection (own psum scope) -----------
            # Dense PE work here overlaps the last batch's DVE/Pool tails.
            qkv_ps.close()
            ps_v = early.enter_context(tc.tile_pool(name="psv", bufs=3, space="PSUM"))
            for j in range(ST):
                vview = v_sb[j][:].rearrange("p (h e) -> p h e", e=128)
                for lo, cw, nh in ((0, 512, 8), (512, 256, 4)):
                    psv = ps_v.tile([128, cw], F32, tag=f"vmm{cw}", name=f"vmm{j}_{lo}")
                    for k in range(KT):
                        nc.tensor.matmul(
                            psv[:], xT[:, k, 128 * j : 128 * j + 128],
                            wvT[:, k, lo : lo + cw],
                            start=(k == 0), stop=False,
                        )
                    nc.tensor.matmul(
                        psv[:], ones_r[:], vbias[:, lo : lo + cw],
                        start=False, stop=True,
                    )
                    nc.scalar.activation(
                        vview[:, lo // 64 : lo // 64 + nh, 64:128], psv[:], AF.Identity, scale=1.0
                    )

        # ---------------- stage 3: attention, software-pipelined ------------
        # PV is organized as per-(head, q-half) accumulation groups over all
        # j (no exp dependencies once the pair's exps exist) and interleaved
        # between the NEXT pair's score matmuls, so the PE has dense filler
        # work instead of chain-waiting on exp slot recycling.
        with ExitStack() as att:
            xpa = att.enter_context(tc.tile_pool(name="attxa", bufs=12))
            xpb = att.enter_context(tc.tile_pool(name="attxb", bufs=12))
            rip = att.enter_context(tc.tile_pool(name="attri", bufs=3))
            ps_sc = att.enter_context(tc.tile_pool(name="pssc", bufs=3, space="PSUM"))
            ps_pv = att.enter_context(tc.tile_pool(name="pspv", bufs=2, space="PSUM"))

            def mk_pv_chunks(p, eAs, eBs):
                """8 closures: two per (head, q-half) group; the second also
                emits the group's recip+normalize epilogue."""
                chunks = []
                for g in range(4):
                    hh, qh = g // 2, g % 2
                    cs = slice(512 * qh, 512 * qh + 512)
                    es = eAs if hh == 0 else eBs
                    h = 2 * p + hh
                    box = {}

                    def first(box=box, h=h, cs=cs, es=es, p=p, g=g):
                        pv = ps_pv.tile([128, 512], F32, tag="pv", name=f"pv{p}_{g}")
                        box["pv"] = pv
                        for j in range(4):
                            vva = v_sb[j][:].rearrange("p (h e) -> p h e", e=128)
                            nc.tensor.matmul(pv[:], vva[:, h, :], es[j][:, cs],
                                             start=(j == 0), stop=False)

                    def second(box=box, h=h, cs=cs, es=es, hh=hh, p=p, g=g):
                        pv = box["pv"]
                        for j in range(4, ST):
                            vva = v_sb[j][:].rearrange("p (h e) -> p h e", e=128)
                            nc.tensor.matmul(pv[:], vva[:, h, :], es[j][:, cs],
                                             start=False, stop=(j == ST - 1))
                        # denominators replicated on psum rows 0:63 via the
                        # leading ones-columns of v; dims on rows 64:127.
                        ri = rip.tile([64, 512], F32, tag="ri", name=f"ri{p}_{g}")
                        nc.vector.reciprocal_approx_fast(ri[:], pv[0:64, :])
                        rows = slice(0, 64) if hh == 0 else slice(64, 128)
                        nc.vector.tensor_mul(outT[p][rows, cs], pv[64:128, :], ri[:])

                    chunks.append(first)
                    chunks.append(second)
                return chunks

            pending_pv = []
            for p in range(KT):
                qt, kt = qk_sb[p], qk_sb[KT + p]
                eAs, eBs = [], []
                for j in range(ST):
                    scA = ps_sc.tile([128, S], F32, tag="sc", name=f"scA{p}_{j}")
                    scB = ps_sc.tile([128, S], F32, tag="sc", name=f"scB{p}_{j}")
                    for h2 in range(2):
                        cs = slice(512 * h2, 512 * h2 + 512)
                        nc.tensor.matmul(
                            scA[:, cs],
                            kt[0:64, 128 * j : 128 * j + 128], qt[0:64, cs],
                            start=True, stop=True,
                        )
                        nc.tensor.matmul(
                            scB[:, cs],
                            kt[64:128, 128 * j : 128 * j + 128], qt[64:128, cs],
                            start=True, stop=True,
                        )
                    # exp split: head A exact on ACT; head B Schraudolph on
                    # DVE (bf16 bit-trick), except 2 of 8 j on ACT to balance
                    eA = xpa.tile([128, S], BF16, tag="expA", name=f"eA{p}_{j}")
                    nc.scalar.activation(eA[:], scA[:], AF.Exp, scale=0.125)
                    eB = xpb.tile([128, S], BF16, tag="expB", name=f"eB{p}_{j}")
                    if j in (3, 7):
                        nc.scalar.activation(eB[:], scB[:], AF.Exp, scale=0.125)
                    else:
                        nc.vector.tensor_scalar(eB[:].bitcast(I16), scB[:], SC_EXP, BC_EXP, OP.mult, OP.add)
                    eAs.append(eA)
                    eBs.append(eB)
                    if pending_pv:
                        pending_pv.pop(0)()
                pending_pv.extend(mk_pv_chunks(p, eAs, eBs))
            for c in pending_pv:
                c()

        # ---------------- stage 4: output projection ------------------------
        with ExitStack() as late:
            yp = late.enter_context(tc.tile_pool(name="yp", bufs=2))
            ps_y = late.enter_context(tc.tile_pool(name="psy", bufs=2, space="PSUM"))
            for mt in range(ST):
                ps = ps_y.tile([128, C], F32, tag="y")
                for cl, cw in ((0, 512), (512, 256)):
                    for k6 in range(KT):
                        nc.tensor.matmul(
                            ps[:, cl : cl + cw],
                            outT[k6][:, 128 * mt : 128 * mt + 128],
                            pwT[:, k6, cl : cl + cw],
                            start=(k6 == 0), stop=False,
                        )
                    nc.tensor.matmul(
                        ps[:, cl : cl + cw], ones_r[:], pbias[:, cl : cl + cw],
                        start=False, stop=True,
                    )
                y = yp.tile([128, C], F32, tag="y_sb")
                nc.scalar.activation(y[:], ps[:], AF.Identity, scale=1.0)
                nc.sync.dma_start(out_d[128 * mt : 128 * mt + 128, :], y[:])


LDW_OPT = False  # walrus LDW-opt rejects bass InstLdweights


def _patch_walrus():
    import concourse.bass_utils as _bu
    if getattr(_bu, "_ldwopt_patched", False):
        return
    _orig = _bu.run_command

    def _patched(cmd, **kw):
        if LDW_OPT and isinstance(cmd, list):
            cmd = ["--enable-ldw-opt=true" if c == "--enable-ldw-opt=false" else c for c in cmd]
        return _orig(cmd, **kw)

    _bu.run_command = _patched
    _bu._ldwopt_patched = True


def build():
    if "nc" in _CACHE:
        return _CACHE["nc"]
    _patch_walrus()
    nc = bacc.Bacc("TRN2", target_bir_lowering=False, debug=False)
    d = {}

    def din(name, shape, dt):
        d[name] = nc.dram_tensor(name, shape, dt, kind="ExternalInput").ap()

    din("xT", [C, S], BF16)
    din("wqkT", [C, 2 * C], BF16)
    din("wvT", [C, C], BF16)
    din("pwT", [C, C], BF16)
    din("bqk", [128, 2 * KT], F32)
    din("vbias", [1, C], BF16)
    din("pbias", [1, C], BF16)
    din("cosq", [128, S], BF16)
    din("sinq", [128, S], BF16)
    din("cosk", [128, S], BF16)
    din("sink", [128, S], BF16)
    din("ebig", [128, 22], BF16)
    din("epsc", [12, 1], F32)
    din("sel", [12, 6 * 128], BF16)
    out_d = nc.dram_tensor("out", [S, C], F32, kind="ExternalOutput").ap()

    with tile.TileContext(nc) as tc:
        _graph(tc, d, out_d)
    nc.compile()
    _CACHE["nc"] = nc
    return nc


def make_in_maps(x, qkv_w, qkv_b, q_norm_w, k_norm_w, proj_w, proj_b):
    shared = _prep_shared(qkv_w, qkv_b, q_norm_w, k_norm_w, proj_w, proj_b)
    x = np.asarray(x, np.float32)
    in_maps = []
    for b in range(NCORES):
        m = dict(shared)
        m["xT"] = np.ascontiguousarray(x[b].T).astype(NP_BF16)
        in_maps.append(m)
    return in_maps


def run(in_maps, trace=False, **kw):
    nc = build()
    return run_bass_kernel_spmd(nc, in_maps, core_ids=list(range(NCORES)), trace=trace, **kw)


def kernel(x, qkv_w, qkv_b, q_norm_w, k_norm_w, proj_w, proj_b):
    in_maps = make_in_maps(x, qkv_w, qkv_b, q_norm_w, k_norm_w, proj_w, proj_b)
    res = run(in_maps)
    return np.stack([np.asarray(res.results[i]["out"]) for i in range(NCORES)]).astype(np.float32)


if __name__ == "__main__":
    rng = np.random.default_rng(0)
    ins = {
        "x": rng.standard_normal((B, S, C)).astype(np.float32),
        "qkv_w": (rng.standard_normal((3 * C, C)) * C**-0.5).astype(np.float32),
        "qkv_b": (rng.standard_normal(3 * C) * 0.02).astype(np.float32),
        "q_norm_w": np.ones(D, np.float32),
        "k_norm_w": np.ones(D, np.float32),
        "proj_w": (rng.standard_normal((C, C)) * C**-0.5).astype(np.float32),
        "proj_b": (rng.standard_normal(C) * 0.02).astype(np.float32),
    }
    y = kernel(**ins)
    print("out", y.shape, y.dtype)
